# revision 5
# baseline (speedup 1.0000x reference)
"""GQA attention (SEQ=2048, DIM=4096, 32 Q heads / 8 KV heads, head_dim=128),
tensor-parallel over heads across 8 NeuronCores.

Each core owns 4 Q heads + 1 KV head: wq/wk/wv split column-wise, wo split
row-wise; each core produces a partial (2048, 4096) output that the host sums
(the all-reduce of row-parallel wo).

Per-core kernel (matmuls on the float32r PE path: full fp32 operand bytes,
tf32-like rounding, 1 cyc/row at free-dim 512 vs 4 cyc/row for plain fp32):
  A) QKV projections: stream xT (dim-major) blocks; Q^T/K^T/V^T accumulate in
     PSUM over the 4096 contraction; RoPE applied on PSUM eviction; V^T
     transposed back to V via PE transposes.
  B) Flash-style attention per (head, 512-query block): S^T = K^T_blk.T @ Q^T
     (keys on partitions), causal staircase mask added on diagonal blocks,
     exp on ACT (scale=1/sqrt(128) folded in), D = ones.T @ expS^T summed over
     key blocks on the PE, O^T = V_blk.T @ expS^T accumulated in PSUM,
     normalized by 1/D (PE broadcast of the reciprocal) on eviction.
  C) out = O^T.T @ wo accumulated over the 4 heads, streamed to DRAM.
"""

import numpy as np

import concourse.bacc as bacc
import concourse.tile as tile
from concourse import mybir
from concourse.bass_utils import run_bass_kernel_spmd

F32 = mybir.dt.float32
F32R = mybir.dt.float32r

DIM = 4096
SEQ = 2048
HEAD_DIM = 128
N_CORES = 8
QH = 4            # q heads per core
QS = QH * HEAD_DIM  # 512: wq column slice per core
NKT = DIM // 128    # 32 contraction tiles
NSB = SEQ // 512    # 4 sequence blocks
SCALE = 1.0 / float(np.sqrt(HEAD_DIM))
NEG = -1e9


def build_nc():
    nc = bacc.Bacc(trn_type="TRN2")

    xT = nc.declare_dram_parameter("xT", [DIM, SEQ], F32R, isOutput=False)
    wq = nc.declare_dram_parameter("wq", [DIM, QS], F32R, isOutput=False)
    wk = nc.declare_dram_parameter("wk", [DIM, HEAD_DIM], F32R, isOutput=False)
    wv = nc.declare_dram_parameter("wv", [DIM, HEAD_DIM], F32R, isOutput=False)
    wo = nc.declare_dram_parameter("wo", [QS, DIM], F32R, isOutput=False)
    cosT = nc.declare_dram_parameter("cosT", [HEAD_DIM, SEQ], F32, isOutput=False)
    sinTs = nc.declare_dram_parameter("sinTs", [HEAD_DIM, SEQ], F32, isOutput=False)
    stair = nc.declare_dram_parameter("stair", [128, 896], F32, isOutput=False)
    ident = nc.declare_dram_parameter("ident", [128, 128], F32R, isOutput=False)
    ones_col = nc.declare_dram_parameter("ones_col", [128, 1], F32R, isOutput=False)
    ones_row = nc.declare_dram_parameter("ones_row", [1, 128], F32R, isOutput=False)
    out = nc.declare_dram_parameter("out", [SEQ, DIM], F32, isOutput=True)

    with tile.TileContext(nc) as tc:
        with (
            tc.tile_pool(name="persist", bufs=1) as persist,
            tc.tile_pool(name="resid", bufs=1) as resid,
        ):
            # small constants
            stair_sb = persist.tile([128, 896], F32)
            nc.sync.dma_start(out=stair_sb, in_=stair[:, :])
            ident_sb = persist.tile([128, 128], F32R)
            nc.sync.dma_start(out=ident_sb, in_=ident[:, :])
            onesc_sb = persist.tile([128, 1], F32R)
            nc.sync.dma_start(out=onesc_sb, in_=ones_col[:, :])
            onesr_sb = persist.tile([1, 128], F32R)
            nc.sync.dma_start(out=onesr_sb, in_=ones_row[:, :])

            # resident activations
            qT = resid.tile([128, QH, SEQ], F32R)      # Q^T per head (d, seq)
            kT = resid.tile([128, SEQ], F32R)          # K^T (d, seq)
            vN = resid.tile([128, SEQ // 128, 128], F32R)  # V natural (keys, d)

            # ---------------- Phase A: projections + RoPE ----------------
            with (
                tc.tile_pool(name="wpool", bufs=1) as wpool,
                tc.tile_pool(name="xpool", bufs=2) as xpool,
                tc.tile_pool(name="cspool", bufs=2) as cspool,
                tc.tile_pool(name="ropetmp", bufs=2) as ropetmp,
                tc.tile_pool(name="vtb", bufs=2) as vtb,
                tc.tile_pool(name="psA", bufs=1, space="PSUM") as psA,
                tc.tile_pool(name="psVT", bufs=2, space="PSUM") as psVT,
            ):
                # resident weights (split DMAs for queue parallelism)
                wq_sb = wpool.tile([128, NKT, QS], F32R)
                wq_r = wq.rearrange("(t p) m -> p t m", p=128)
                for c in range(8):
                    nc.sync.dma_start(
                        out=wq_sb[:, c * 4:(c + 1) * 4, :],
                        in_=wq_r[:, c * 4:(c + 1) * 4, :],
                    )
                wk_sb = wpool.tile([128, NKT, HEAD_DIM], F32R)
                wk_r = wk.rearrange("(t p) m -> p t m", p=128)
                wv_sb = wpool.tile([128, NKT, HEAD_DIM], F32R)
                wv_r = wv.rearrange("(t p) m -> p t m", p=128)
                for c in range(2):
                    nc.sync.dma_start(
                        out=wk_sb[:, c * 16:(c + 1) * 16, :],
                        in_=wk_r[:, c * 16:(c + 1) * 16, :],
                    )
                    nc.sync.dma_start(
                        out=wv_sb[:, c * 16:(c + 1) * 16, :],
                        in_=wv_r[:, c * 16:(c + 1) * 16, :],
                    )

                xT_r = xT.rearrange("(t p) s -> p t s", p=128)

                for sb in range(NSB):
                    ss = slice(sb * 512, (sb + 1) * 512)
                    # PSUM accumulators for this seq block
                    q_ps = [psA.tile([128, 512], F32, tag=f"qps{h}", name=f"qps{h}")
                            for h in range(QH)]
                    k_ps = psA.tile([128, 512], F32, tag="kps")
                    v_ps = psA.tile([128, 512], F32, tag="vps")

                    for g in range(8):  # super-tiles of 4 k-tiles (1 MiB DMAs)
                        xt = xpool.tile([128, 4, 512], F32R, tag="xt")
                        nc.sync.dma_start(
                            out=xt, in_=xT_r[:, g * 4:(g + 1) * 4, ss]
                        )
                        for i in range(4):
                            kt = g * 4 + i
                            st = (kt == 0)
                            sp = (kt == NKT - 1)
                            for h in range(QH):
                                nc.tensor.matmul(
                                    q_ps[h],
                                    wq_sb[:, kt, h * 128:(h + 1) * 128],
                                    xt[:, i, :],
                                    start=st, stop=sp,
                                )
                            nc.tensor.matmul(
                                k_ps, wk_sb[:, kt, :], xt[:, i, :],
                                start=st, stop=sp,
                            )
                            nc.tensor.matmul(
                                v_ps, wv_sb[:, kt, :], xt[:, i, :],
                                start=st, stop=sp,
                            )

                    # RoPE tables for this block
                    cos_t = cspool.tile([128, 512], F32, tag="cos")
                    nc.sync.dma_start(out=cos_t, in_=cosT[:, ss])
                    sin_t = cspool.tile([128, 512], F32, tag="sin")
                    nc.sync.dma_start(out=sin_t, in_=sinTs[:, ss])

                    def rope(dst, src_ps):
                        # dst = src*cos + rot_half(src)*sin_signed
                        t = ropetmp.tile([128, 512], F32, tag="t", name="t")
                        u = ropetmp.tile([128, 512], F32, tag="u", name="u")
                        nc.vector.tensor_mul(t, src_ps, cos_t)
                        nc.vector.tensor_mul(
                            u[0:64, :], src_ps[64:128, :], sin_t[0:64, :]
                        )
                        nc.vector.tensor_mul(
                            u[64:128, :], src_ps[0:64, :], sin_t[64:128, :]
                        )
                        nc.vector.tensor_add(dst, t, u)

                    for h in range(QH):
                        rope(qT[:, h, ss], q_ps[h])
                    rope(kT[:, ss], k_ps)

                    # V^T -> V via PE transposes
                    vt_sb = vtb.tile([128, 512], F32R, tag="vt")
                    nc.vector.tensor_copy(vt_sb, v_ps)
                    for j in range(4):
                        vt_ps = psVT.tile([128, 128], F32R, tag="vtp", name="vtp")
                        nc.tensor.transpose(
                            vt_ps, vt_sb[:, j * 128:(j + 1) * 128], ident_sb
                        )
                        nc.vector.tensor_copy(vN[:, sb * 4 + j, :], vt_ps)

            # ---------------- Phase B/C: attention + out projection ----------------
            with (
                tc.tile_pool(name="wopool", bufs=1) as wopool,
                tc.tile_pool(name="expp", bufs=8) as expp,
                tc.tile_pool(name="otp", bufs=2) as otp,
                tc.tile_pool(name="dsmall", bufs=2) as dsmall,
                tc.tile_pool(name="bcp", bufs=2) as bcp,
                tc.tile_pool(name="outev", bufs=3) as outev,
                tc.tile_pool(name="psS", bufs=2, space="PSUM") as psS,
                tc.tile_pool(name="psD", bufs=1, space="PSUM") as psD,
                tc.tile_pool(name="psOT", bufs=2, space="PSUM") as psOT,
                tc.tile_pool(name="psBC", bufs=1, space="PSUM") as psBC,
                tc.tile_pool(name="psC", bufs=2, space="PSUM") as psC,
            ):
                wo_sb = wopool.tile([128, QH, DIM], F32R)
                wo_r = wo.rearrange("(h p) n -> p h n", p=128)
                for h in range(QH):
                    for c in range(2):
                        nc.sync.dma_start(
                            out=wo_sb[:, h, c * 2048:(c + 1) * 2048],
                            in_=wo_r[:, h, c * 2048:(c + 1) * 2048],
                        )

                for qb in range(NSB):
                    qs = slice(qb * 512, (qb + 1) * 512)
                    n_kb = 4 * qb + 4
                    ot_sb = [None] * QH
                    for h in range(QH):
                        d_ps = psD.tile([1, 512], F32, tag="dps", name="dps")
                        ot_ps = psOT.tile([128, 512], F32, tag="otps", name="otps")
                        for kb in range(n_kb):
                            s_ps = psS.tile([128, 512], F32, tag="sps", name="sps")
                            nc.tensor.matmul(
                                s_ps,
                                kT[:, kb * 128:(kb + 1) * 128],
                                qT[:, h, qs],
                                start=True, stop=True,
                            )
                            j = kb - 4 * qb
                            if j >= 0:  # diagonal block: causal mask
                                nc.vector.tensor_add(
                                    s_ps, s_ps,
                                    stair_sb[:, 384 - 128 * j:896 - 128 * j],
                                )
                            es = expp.tile([128, 512], F32R, tag="es", name="es")
                            nc.scalar.activation(
                                es, s_ps, mybir.ActivationFunctionType.Exp,
                                scale=SCALE,
                            )
                            nc.tensor.matmul(
                                d_ps, onesc_sb, es,
                                start=(kb == 0), stop=(kb == n_kb - 1),
                            )
                            nc.tensor.matmul(
                                ot_ps, vN[:, kb, :], es,
                                start=(kb == 0), stop=(kb == n_kb - 1),
                            )
                        # normalize: O^T * (1/D) broadcast across partitions
                        rd = dsmall.tile([1, 512], F32R, tag="rd", name="rd")
                        with nc.allow_low_precision("f32r reciprocal for PE bcast"):
                            nc.vector.reciprocal(rd, d_ps)
                        bc_ps = psBC.tile([128, 512], F32, tag="bc", name="bc")
                        nc.tensor.matmul(
                            bc_ps, onesr_sb, rd, start=True, stop=True
                        )
                        bc_sb = bcp.tile([128, 512], F32, tag="bcsb", name="bcsb")
                        nc.scalar.copy(bc_sb, bc_ps)
                        ot = otp.tile([128, 512], F32R, tag=f"ot{h}", name=f"ot{h}")
                        nc.vector.tensor_mul(ot, ot_ps, bc_sb)
                        ot_sb[h] = ot

                    # Phase C for this query block
                    for qc in range(4):
                        for nb in range(8):
                            o_ps = psC.tile([128, 512], F32, tag="ops", name="ops")
                            for h in range(QH):
                                nc.tensor.matmul(
                                    o_ps,
                                    ot_sb[h][:, qc * 128:(qc + 1) * 128],
                                    wo_sb[:, h, nb * 512:(nb + 1) * 512],
                                    start=(h == 0), stop=(h == QH - 1),
                                )
                            ob = outev.tile([128, 512], F32, tag="ob", name="ob")
                            nc.vector.tensor_copy(ob, o_ps)
                            nc.sync.dma_start(
                                out=out[qb * 512 + qc * 128:
                                        qb * 512 + (qc + 1) * 128,
                                        nb * 512:(nb + 1) * 512],
                                in_=ob,
                            )
    nc.finalize()
    return nc


_NC_CACHE = {}


def _get_nc():
    if "nc" not in _NC_CACHE:
        _NC_CACHE["nc"] = build_nc()
    return _NC_CACHE["nc"]


def _host_prep(x, cos, sin, mask, wq, wk, wv, wo):
    xT = np.ascontiguousarray(x[0].T.astype(np.float32))
    cosT = np.ascontiguousarray(cos[:, 0, :].T.astype(np.float32))
    sinT = sin[:, 0, :].T.astype(np.float32)
    sinTs = np.ascontiguousarray(
        np.concatenate([-sinT[:64], sinT[64:]], axis=0)
    )
    rr = np.arange(128, dtype=np.int64)[:, None]
    cc = np.arange(896, dtype=np.int64)[None, :]
    stair = np.where(rr <= cc - 384, 0.0, NEG).astype(np.float32)
    ident = np.eye(128, dtype=np.float32)
    ones_col = np.ones((128, 1), dtype=np.float32)
    ones_row = np.ones((1, 128), dtype=np.float32)

    in_maps = []
    for i in range(N_CORES):
        in_maps.append({
            "xT": xT,
            "wq": np.ascontiguousarray(wq[:, i * QS:(i + 1) * QS]),
            "wk": np.ascontiguousarray(wk[:, i * 128:(i + 1) * 128]),
            "wv": np.ascontiguousarray(wv[:, i * 128:(i + 1) * 128]),
            "wo": np.ascontiguousarray(wo[i * QS:(i + 1) * QS, :]),
            "cosT": cosT,
            "sinTs": sinTs,
            "stair": stair,
            "ident": ident,
            "ones_col": ones_col,
            "ones_row": ones_row,
        })
    return in_maps


def kernel(x, cos, sin, mask, wq, wk, wv, wo, _trace=False, _trace_kwargs=None):
    nc = _get_nc()
    in_maps = _host_prep(x, cos, sin, mask, wq, wk, wv, wo)
    res = run_bass_kernel_spmd(
        nc, in_maps, list(range(N_CORES)), trace=_trace,
        **(_trace_kwargs or {}),
    )
    partials = [res.results[i]["out"] for i in range(N_CORES)]
    full = np.sum(np.stack(partials, axis=0), axis=0, dtype=np.float64)
    out = full.astype(np.float32)[None, :, :]
    if _trace:
        return out, res
    return out


# revision 6
# speedup vs baseline: 1.0289x; 1.0289x over previous
"""GQA attention (SEQ=2048, DIM=4096, 32 Q heads / 8 KV heads, head_dim=128),
tensor-parallel over heads across 8 NeuronCores.

Each core owns 4 Q heads + 1 KV head: wq/wk/wv split column-wise, wo split
row-wise; each core produces a partial (2048, 4096) output that the host sums
(the all-reduce of row-parallel wo).

Per-core kernel (matmuls on the float32r PE path: full fp32 operand bytes,
tf32-like rounding, 1 cyc/row at free-dim 512 vs 4 cyc/row for plain fp32):
  A) QKV projections: stream xT (dim-major) blocks; Q^T/K^T/V^T accumulate in
     PSUM over the 4096 contraction; RoPE applied on PSUM eviction; V^T
     transposed back to V via PE transposes.
  B) Flash-style attention per (head, 512-query block): S^T = K^T_blk.T @ Q^T
     (keys on partitions), causal staircase mask added on diagonal blocks,
     exp on ACT (scale=1/sqrt(128) folded in), D = ones.T @ expS^T summed over
     key blocks on the PE, O^T = V_blk.T @ expS^T accumulated in PSUM,
     normalized by 1/D (PE broadcast of the reciprocal) on eviction.
  C) out = O^T.T @ wo accumulated over the 4 heads, streamed to DRAM.
"""

import numpy as np

import concourse.bacc as bacc
import concourse.tile as tile
from concourse import mybir
from concourse.bass_utils import run_bass_kernel_spmd

F32 = mybir.dt.float32
F32R = mybir.dt.float32r

DIM = 4096
SEQ = 2048
HEAD_DIM = 128
N_CORES = 8
QH = 4            # q heads per core
QS = QH * HEAD_DIM  # 512: wq column slice per core
NKT = DIM // 128    # 32 contraction tiles
NSB = SEQ // 512    # 4 sequence blocks
SCALE = 1.0 / float(np.sqrt(HEAD_DIM))
NEG = -1e9


def build_nc():
    nc = bacc.Bacc(trn_type="TRN2")

    xT = nc.declare_dram_parameter("xT", [DIM, SEQ], F32R, isOutput=False)
    wq = nc.declare_dram_parameter("wq", [DIM, QS], F32R, isOutput=False)
    wk = nc.declare_dram_parameter("wk", [DIM, HEAD_DIM], F32R, isOutput=False)
    wv = nc.declare_dram_parameter("wv", [DIM, HEAD_DIM], F32R, isOutput=False)
    wo = nc.declare_dram_parameter("wo", [QS, DIM], F32R, isOutput=False)
    cosT = nc.declare_dram_parameter("cosT", [HEAD_DIM, SEQ], F32, isOutput=False)
    sinTs = nc.declare_dram_parameter("sinTs", [HEAD_DIM, SEQ], F32, isOutput=False)
    stair = nc.declare_dram_parameter("stair", [128, 896], F32, isOutput=False)
    ident = nc.declare_dram_parameter("ident", [128, 128], F32R, isOutput=False)
    ones_col = nc.declare_dram_parameter("ones_col", [128, 1], F32R, isOutput=False)
    ones_row = nc.declare_dram_parameter("ones_row", [1, 128], F32R, isOutput=False)
    out = nc.declare_dram_parameter("out", [SEQ, DIM], F32, isOutput=True)

    with tile.TileContext(nc) as tc:
        with (
            tc.tile_pool(name="persist", bufs=1) as persist,
            tc.tile_pool(name="resid", bufs=1) as resid,
        ):
            # small constants
            stair_sb = persist.tile([128, 896], F32)
            nc.sync.dma_start(out=stair_sb, in_=stair[:, :])
            ident_sb = persist.tile([128, 128], F32R)
            nc.sync.dma_start(out=ident_sb, in_=ident[:, :])
            onesc_sb = persist.tile([128, 1], F32R)
            nc.sync.dma_start(out=onesc_sb, in_=ones_col[:, :])
            onesr_sb = persist.tile([1, 128], F32R)
            nc.sync.dma_start(out=onesr_sb, in_=ones_row[:, :])

            # resident activations
            qT = resid.tile([128, QH, SEQ], F32R)      # Q^T per head (d, seq)
            kT = resid.tile([128, SEQ], F32R)          # K^T (d, seq)
            vN = resid.tile([128, SEQ // 128, 128], F32R)  # V natural (keys, d)

            # ---------------- Phase A: projections + RoPE ----------------
            with (
                tc.tile_pool(name="wpool", bufs=1) as wpool,
                tc.tile_pool(name="xpool", bufs=2) as xpool,
                tc.tile_pool(name="cspool", bufs=2) as cspool,
                tc.tile_pool(name="ropetmp", bufs=2) as ropetmp,
                tc.tile_pool(name="vtb", bufs=2) as vtb,
                tc.tile_pool(name="psA", bufs=1, space="PSUM") as psA,
                tc.tile_pool(name="psVT", bufs=2, space="PSUM") as psVT,
            ):
                # resident weights: per-4kt chunk tiles so the first matmuls
                # only wait on their own 1 MiB DMA, not the whole weight load
                wq_r = wq.rearrange("(t p) m -> p t m", p=128)
                wk_r = wk.rearrange("(t p) m -> p t m", p=128)
                wv_r = wv.rearrange("(t p) m -> p t m", p=128)
                wq_cs, wk_cs, wv_cs = [], [], []
                for c in range(8):
                    wqc = wpool.tile([128, 4, QS], F32R, name=f"wqc{c}")
                    nc.sync.dma_start(out=wqc, in_=wq_r[:, c * 4:(c + 1) * 4, :])
                    wq_cs.append(wqc)
                    wkc = wpool.tile([128, 4, HEAD_DIM], F32R, name=f"wkc{c}")
                    nc.sync.dma_start(out=wkc, in_=wk_r[:, c * 4:(c + 1) * 4, :])
                    wk_cs.append(wkc)
                    wvc = wpool.tile([128, 4, HEAD_DIM], F32R, name=f"wvc{c}")
                    nc.sync.dma_start(out=wvc, in_=wv_r[:, c * 4:(c + 1) * 4, :])
                    wv_cs.append(wvc)

                xT_r = xT.rearrange("(t p) s -> p t s", p=128)

                for sb in range(NSB):
                    ss = slice(sb * 512, (sb + 1) * 512)
                    # PSUM accumulators for this seq block
                    q_ps = [psA.tile([128, 512], F32, tag=f"qps{h}", name=f"qps{h}")
                            for h in range(QH)]
                    k_ps = psA.tile([128, 512], F32, tag="kps")
                    v_ps = psA.tile([128, 512], F32, tag="vps")

                    for g in range(8):  # super-tiles of 4 k-tiles (1 MiB DMAs)
                        xt = xpool.tile([128, 4, 512], F32R, tag="xt")
                        nc.sync.dma_start(
                            out=xt, in_=xT_r[:, g * 4:(g + 1) * 4, ss]
                        )
                        for i in range(4):
                            kt = g * 4 + i
                            st = (kt == 0)
                            sp = (kt == NKT - 1)
                            for h in range(QH):
                                nc.tensor.matmul(
                                    q_ps[h],
                                    wq_cs[g][:, i, h * 128:(h + 1) * 128],
                                    xt[:, i, :],
                                    start=st, stop=sp,
                                )
                            nc.tensor.matmul(
                                k_ps, wk_cs[g][:, i, :], xt[:, i, :],
                                start=st, stop=sp,
                            )
                            nc.tensor.matmul(
                                v_ps, wv_cs[g][:, i, :], xt[:, i, :],
                                start=st, stop=sp,
                            )

                    # RoPE tables for this block
                    cos_t = cspool.tile([128, 512], F32, tag="cos")
                    nc.sync.dma_start(out=cos_t, in_=cosT[:, ss])
                    sin_t = cspool.tile([128, 512], F32, tag="sin")
                    nc.sync.dma_start(out=sin_t, in_=sinTs[:, ss])

                    def rope(dst, src_ps):
                        # dst = src*cos + rot_half(src)*sin_signed
                        t = ropetmp.tile([128, 512], F32, tag="t", name="t")
                        u = ropetmp.tile([128, 512], F32, tag="u", name="u")
                        nc.vector.tensor_mul(t, src_ps, cos_t)
                        nc.vector.tensor_mul(
                            u[0:64, :], src_ps[64:128, :], sin_t[0:64, :]
                        )
                        nc.vector.tensor_mul(
                            u[64:128, :], src_ps[0:64, :], sin_t[64:128, :]
                        )
                        nc.vector.tensor_add(dst, t, u)

                    # V^T -> V via PE transposes (ACT evicts, keeping DVE free
                    # for RoPE; issued first so the PE transposes overlap ropes)
                    vt_sb = vtb.tile([128, 512], F32R, tag="vt")
                    nc.scalar.copy(vt_sb, v_ps)
                    for j in range(4):
                        vt_ps = psVT.tile([128, 128], F32R, tag="vtp", name="vtp")
                        nc.tensor.transpose(
                            vt_ps, vt_sb[:, j * 128:(j + 1) * 128], ident_sb
                        )
                        nc.scalar.copy(vN[:, sb * 4 + j, :], vt_ps)

                    for h in range(QH):
                        rope(qT[:, h, ss], q_ps[h])
                    rope(kT[:, ss], k_ps)

            # ---------------- Phase B/C: attention + out projection ----------------
            with (
                tc.tile_pool(name="wopool", bufs=1) as wopool,
                tc.tile_pool(name="expp", bufs=8) as expp,
                tc.tile_pool(name="otp", bufs=2) as otp,
                tc.tile_pool(name="dsmall", bufs=2) as dsmall,
                tc.tile_pool(name="bcp", bufs=2) as bcp,
                tc.tile_pool(name="outev", bufs=3) as outev,
                tc.tile_pool(name="psS", bufs=3, space="PSUM") as psS,
                tc.tile_pool(name="psD", bufs=1, space="PSUM") as psD,
                tc.tile_pool(name="psOT", bufs=2, space="PSUM") as psOT,
                tc.tile_pool(name="psC", bufs=2, space="PSUM") as psC,
            ):
                wo_sb = wopool.tile([128, QH, DIM], F32R)
                wo_r = wo.rearrange("(h p) n -> p h n", p=128)
                for h in range(QH):
                    for c in range(2):
                        nc.sync.dma_start(
                            out=wo_sb[:, h, c * 2048:(c + 1) * 2048],
                            in_=wo_r[:, h, c * 2048:(c + 1) * 2048],
                        )

                LAG = 4  # D/AV matmuls trail the score stream by LAG blocks
                for qb in range(NSB):
                    qs = slice(qb * 512, (qb + 1) * 512)
                    n_kb = 4 * qb + 4
                    ot_sb = [None] * QH
                    for h in range(QH):
                        d_ps = psD.tile([1, 512], F32, tag="dps", name="dps")
                        ot_ps = psOT.tile([128, 512], F32, tag="otps", name="otps")
                        ess = [None] * n_kb

                        def drain(kb):
                            nc.tensor.matmul(
                                d_ps, onesc_sb, ess[kb],
                                start=(kb == 0), stop=(kb == n_kb - 1),
                            )
                            nc.tensor.matmul(
                                ot_ps, vN[:, kb, :], ess[kb],
                                start=(kb == 0), stop=(kb == n_kb - 1),
                            )

                        for kb in range(n_kb):
                            s_ps = psS.tile([128, 512], F32, tag="sps", name="sps")
                            nc.tensor.matmul(
                                s_ps,
                                kT[:, kb * 128:(kb + 1) * 128],
                                qT[:, h, qs],
                                start=True, stop=True,
                            )
                            j = kb - 4 * qb
                            if j >= 0:  # diagonal block: causal mask
                                nc.vector.tensor_add(
                                    s_ps, s_ps,
                                    stair_sb[:, 384 - 128 * j:896 - 128 * j],
                                )
                            es = expp.tile([128, 512], F32R, tag="es", name="es")
                            nc.scalar.activation(
                                es, s_ps, mybir.ActivationFunctionType.Exp,
                                scale=SCALE,
                            )
                            ess[kb] = es
                            if kb >= LAG:
                                drain(kb - LAG)
                        for kb in range(max(0, n_kb - LAG), n_kb):
                            drain(kb)
                        # normalize: O^T * (1/D) broadcast across partitions
                        rd = dsmall.tile([1, 512], F32R, tag="rd", name="rd")
                        with nc.allow_low_precision("f32r reciprocal for PE bcast"):
                            nc.vector.reciprocal(rd, d_ps)
                        bc_ps = psS.tile([128, 512], F32, tag="sps", name="bc")
                        nc.tensor.matmul(
                            bc_ps, onesr_sb, rd, start=True, stop=True
                        )
                        bc_sb = bcp.tile([128, 512], F32, tag="bcsb", name="bcsb")
                        nc.scalar.copy(bc_sb, bc_ps)
                        ot = otp.tile([128, 512], F32R, tag=f"ot{h}", name=f"ot{h}")
                        nc.vector.tensor_mul(ot, ot_ps, bc_sb)
                        ot_sb[h] = ot

                    # Phase C for this query block
                    for qc in range(4):
                        for nb in range(8):
                            o_ps = psC.tile([128, 512], F32, tag="ops", name="ops")
                            for h in range(QH):
                                nc.tensor.matmul(
                                    o_ps,
                                    ot_sb[h][:, qc * 128:(qc + 1) * 128],
                                    wo_sb[:, h, nb * 512:(nb + 1) * 512],
                                    start=(h == 0), stop=(h == QH - 1),
                                )
                            ob = outev.tile([128, 512], F32, tag="ob", name="ob")
                            nc.vector.tensor_copy(ob, o_ps)
                            nc.sync.dma_start(
                                out=out[qb * 512 + qc * 128:
                                        qb * 512 + (qc + 1) * 128,
                                        nb * 512:(nb + 1) * 512],
                                in_=ob,
                            )
    nc.finalize()
    return nc


_NC_CACHE = {}


def _get_nc():
    if "nc" not in _NC_CACHE:
        _NC_CACHE["nc"] = build_nc()
    return _NC_CACHE["nc"]


def _host_prep(x, cos, sin, mask, wq, wk, wv, wo):
    xT = np.ascontiguousarray(x[0].T.astype(np.float32))
    cosT = np.ascontiguousarray(cos[:, 0, :].T.astype(np.float32))
    sinT = sin[:, 0, :].T.astype(np.float32)
    sinTs = np.ascontiguousarray(
        np.concatenate([-sinT[:64], sinT[64:]], axis=0)
    )
    rr = np.arange(128, dtype=np.int64)[:, None]
    cc = np.arange(896, dtype=np.int64)[None, :]
    stair = np.where(rr <= cc - 384, 0.0, NEG).astype(np.float32)
    ident = np.eye(128, dtype=np.float32)
    ones_col = np.ones((128, 1), dtype=np.float32)
    ones_row = np.ones((1, 128), dtype=np.float32)

    in_maps = []
    for i in range(N_CORES):
        in_maps.append({
            "xT": xT,
            "wq": np.ascontiguousarray(wq[:, i * QS:(i + 1) * QS]),
            "wk": np.ascontiguousarray(wk[:, i * 128:(i + 1) * 128]),
            "wv": np.ascontiguousarray(wv[:, i * 128:(i + 1) * 128]),
            "wo": np.ascontiguousarray(wo[i * QS:(i + 1) * QS, :]),
            "cosT": cosT,
            "sinTs": sinTs,
            "stair": stair,
            "ident": ident,
            "ones_col": ones_col,
            "ones_row": ones_row,
        })
    return in_maps


def kernel(x, cos, sin, mask, wq, wk, wv, wo, _trace=False, _trace_kwargs=None):
    nc = _get_nc()
    in_maps = _host_prep(x, cos, sin, mask, wq, wk, wv, wo)
    res = run_bass_kernel_spmd(
        nc, in_maps, list(range(N_CORES)), trace=_trace,
        **(_trace_kwargs or {}),
    )
    partials = [res.results[i]["out"] for i in range(N_CORES)]
    full = np.sum(np.stack(partials, axis=0), axis=0, dtype=np.float64)
    out = full.astype(np.float32)[None, :, :]
    if _trace:
        return out, res
    return out
